# revision 6
# baseline (speedup 1.0000x reference)
"""Trainium2 Bass kernel for nn_PreconditionerNet (5-layer conv stack + PReLU + tril mask).

Strategy
--------
8 cores = 4 images x 2 row-halves (halo recompute, no collectives). Each core
runs an identical program over a 132-row input window and produces 128 rows of
the final 256x256 output for its (image, half).

Inside a core the whole conv stack is fused into one pass over 2-row "ticks":
activations live only in SBUF rolling windows (3-row tiles), so DRAM traffic is
just the tiny input window + output. Convs are computed as fp32r matmuls
(channels on partitions, 2x2 kernels = 4 shifted accumulating matmuls); PReLU
epilogues run on the scalar engine as single Prelu ops (PSUM -> SBUF).

conv1 (1x1, 1->64) + PReLU is folded into conv2 on the host: with b1 == 0,
PReLU(w1_c * x) = A[c,0]*relu(x) + A[c,1]*min(x,0), so conv2 over the 64
PReLU'd channels becomes a K=8 matmul over a host-built "Vstack" tensor
(4 spatial shifts x 2 half-rectified channels) with folded weights W2A.
"""

import os
import sys

sys.path.insert(0, "/opt/trn_rl_repo")

import numpy as np

import concourse.bacc as bacc
import concourse.mybir as mybir
import concourse.tile as tile
from concourse.bass_utils import run_bass_kernel_spmd

f32 = mybir.dt.float32
f32r = mybir.dt.float32r
PRELU = mybir.ActivationFunctionType.Prelu

# Geometry (hardcoded for x: [4, 256, 256, 1])
B, N = 4, 256
HALF = 128
XROWS = 132          # input window rows per core
VROWS = 131          # Vstack rows (= L2 blocks * 2 - 1)
R2, R3, R4, R5 = 131, 130, 129, 128   # act rows per layer window
T2 = 66              # L2 2-row blocks (last is single-row)
S2, S3, S4 = 256, 258, 258            # stored column strides (act2/act3/act4)
NTICKS = 67

_CACHE = {}
LAST_RESULTS = None


def _build_program(alphas):
    a2, a3, a4, a5 = alphas
    nc = bacc.Bacc("TRN2", target_bir_lowering=False, debug=False)

    # --- DRAM tensors ---
    vst_d = nc.dram_tensor("vstack", [8, VROWS * 256], f32, kind="ExternalInput")
    mask_d = nc.dram_tensor("mask", [1, HALF * 256], f32, kind="ExternalInput")
    w2a_d = nc.dram_tensor("w2a", [8, 256], f32, kind="ExternalInput")
    w3_d = nc.dram_tensor("w3", [4, 256, 512], f32, kind="ExternalInput")
    w4_d = nc.dram_tensor("w4", [4, 512, 256], f32, kind="ExternalInput")
    w5_d = nc.dram_tensor("w5", [4, 256, 64], f32, kind="ExternalInput")
    w6_d = nc.dram_tensor("w6", [64, 1], f32, kind="ExternalInput")
    b2_d = nc.dram_tensor("b2", [256, 1], f32, kind="ExternalInput")
    b3_d = nc.dram_tensor("b3", [512, 1], f32, kind="ExternalInput")
    b4_d = nc.dram_tensor("b4", [256, 1], f32, kind="ExternalInput")
    b5_d = nc.dram_tensor("b5", [64, 1], f32, kind="ExternalInput")
    out_d = nc.dram_tensor("out", [1, HALF * 256], f32, kind="ExternalOutput")

    with tile.TileContext(nc) as tc:
        with (
            tc.tile_pool(name="wstage", bufs=2) as wstage,
            tc.tile_pool(name="weights", bufs=1) as wpool,
            tc.tile_pool(name="vs", bufs=3) as vspool,
            tc.tile_pool(name="a2", bufs=3) as a2pool,
            tc.tile_pool(name="a3", bufs=3) as a3pool,
            tc.tile_pool(name="a4", bufs=3) as a4pool,
            tc.tile_pool(name="a5", bufs=3) as a5pool,
            tc.tile_pool(name="outst", bufs=3) as outpool,
            tc.tile_pool(name="ps2", bufs=2, space="PSUM") as ps2pool,
            tc.tile_pool(name="ps3", bufs=2, space="PSUM") as ps3pool,
            tc.tile_pool(name="ps4", bufs=2, space="PSUM") as ps4pool,
            tc.tile_pool(name="ps5", bufs=1, space="PSUM") as ps5pool,
            tc.tile_pool(name="ps6", bufs=1, space="PSUM") as ps6pool,
        ):
            # ---- load + convert weights to f32r ----
            def load_wr(src_ap, p, cols, tag):
                st = wstage.tile([128, 512], f32, tag="wstage")
                nc.sync.dma_start(st[:p, :cols], src_ap)
                wr = wpool.tile([p, cols], f32r, tag=tag)
                nc.vector.tensor_copy(wr[:], st[:p, :cols])
                return wr

            w2a_r = load_wr(w2a_d.ap(), 8, 256, "w2a")
            w3_r = [[load_wr(w3_d.ap()[s, kt * 128:(kt + 1) * 128, :], 128, 512,
                             f"w3_{s}_{kt}") for kt in range(2)] for s in range(4)]
            w4_r = [[load_wr(w4_d.ap()[s, kt * 128:(kt + 1) * 128, :], 128, 256,
                             f"w4_{s}_{kt}") for kt in range(4)] for s in range(4)]
            w5_r = [[load_wr(w5_d.ap()[s, kt * 128:(kt + 1) * 128, :], 128, 64,
                             f"w5_{s}_{kt}") for kt in range(2)] for s in range(4)]
            w6_r = load_wr(w6_d.ap(), 64, 1, "w6")

            def load_bias(src_ap, p, tag):
                bt = wpool.tile([p, 1], f32, tag=tag)
                nc.sync.dma_start(bt[:], src_ap)
                return bt

            b2_t = [load_bias(b2_d.ap()[mt * 128:(mt + 1) * 128, :], 128, f"b2_{mt}")
                    for mt in range(2)]
            b3_t = [load_bias(b3_d.ap()[mt * 128:(mt + 1) * 128, :], 128, f"b3_{mt}")
                    for mt in range(4)]
            b4_t = [load_bias(b4_d.ap()[mt * 128:(mt + 1) * 128, :], 128, f"b4_{mt}")
                    for mt in range(2)]
            b5_t = load_bias(b5_d.ap(), 64, "b5")

            # persistent zeros tile (f32r) for margin zeroing
            zf = wpool.tile([128, 8], f32, tag="zf")
            nc.vector.memset(zf[:], 0.0)
            zr = wpool.tile([128, 8], f32r, tag="zr")
            nc.vector.tensor_copy(zr[:], zf[:])
            zv = zr[:].rearrange("p (r c) -> p r c", c=4)  # [128, 2, 4]

            # rolling-window tiles, keyed by block index
            a2_tiles, a3_tiles, a4_tiles, a5_tiles = {}, {}, {}, {}

            def rows(view):
                return view.rearrange("p (r c) -> p r c", r=3)

            for t in range(NTICKS):
                # ---------------- L2: block t -> act2 ----------------
                if t < T2:
                    cnt = 2 if t < T2 - 1 else 1
                    vs_f = vspool.tile([8, 512], f32, tag="vsf")
                    nc.sync.dma_start(
                        vs_f[:, :cnt * 256],
                        vst_d.ap()[:, 2 * t * 256:(2 * t + cnt) * 256])
                    vs_r = vspool.tile([8, 512], f32r, tag="vsr")
                    nc.vector.tensor_copy(vs_r[:, :cnt * 256], vs_f[:, :cnt * 256])
                    a2_tiles[t] = []
                    for mt in range(2):
                        ps = ps2pool.tile([128, 512], f32, tag="ps2")
                        nc.tensor.matmul(ps[:, :cnt * 256],
                                         w2a_r[:, mt * 128:(mt + 1) * 128],
                                         vs_r[:, :cnt * 256],
                                         start=True, stop=True)
                        at = a2pool.tile([128, 3 * S2], f32r, tag=f"a2k{mt}")
                        a2_tiles[t].append(at)
                        nc.scalar.activation(at[:, :cnt * 256], ps[:, :cnt * 256],
                                             PRELU, bias=b2_t[mt][:], scale=1.0,
                                             alpha=a2)
                        if t >= 1:
                            # duplicate first row into previous tile's row 2
                            nc.vector.tensor_copy(
                                a2_tiles[t - 1][mt][:, 2 * S2:2 * S2 + 256],
                                at[:, 0:256])

                # ---------------- L3: block t-1 -> act3 ----------------
                u = t - 1
                if 0 <= u <= 64:
                    a3_tiles[u] = []
                    for mt in range(4):
                        ps = ps3pool.tile([128, 508], f32, tag="ps3")
                        i = 0
                        for dy in range(2):
                            for dx in range(2):
                                for kt in range(2):
                                    rhs = rows(a2_tiles[u][kt][:])[
                                        :, dy:dy + 2, dx:dx + 254]
                                    nc.tensor.matmul(
                                        ps[:],
                                        w3_r[dy * 2 + dx][kt][:, mt * 128:(mt + 1) * 128],
                                        rhs, start=(i == 0), stop=(i == 7))
                                    i += 1
                        at = a3pool.tile([128, 3 * S3], f32r, tag=f"a3k{mt}")
                        a3_tiles[u].append(at)
                        av = at[:].rearrange("p (r c) -> p r c", c=S3)
                        # zero margins (cols 0, 255, 256) for rows 0..1
                        nc.vector.tensor_copy(av[:, 0:2, 0:1], zv[:, 0:2, 0:1])
                        nc.vector.tensor_copy(av[:, 0:2, 255:257], zv[:, 0:2, 0:2])
                        nc.scalar.activation(av[:, 0:2, 1:255],
                                             ps[:].rearrange("p (r c) -> p r c", c=254),
                                             PRELU, bias=b3_t[mt][:], scale=1.0,
                                             alpha=a3)
                        if u >= 1:
                            nc.vector.tensor_copy(
                                a3_tiles[u - 1][mt][:, 2 * S3:3 * S3],
                                at[:, 0:S3])

                # ---------------- L4: block t-2 -> act4 ----------------
                v = t - 2
                if 0 <= v <= 64:
                    cnt = 2 if v < 64 else 1
                    a4_tiles[v] = []
                    for mt in range(2):
                        ps = ps4pool.tile([128, 512], f32, tag="ps4")
                        i = 0
                        for dy in range(2):
                            for dx in range(2):
                                for kt in range(4):
                                    rhs = a3_tiles[v][kt][:].rearrange(
                                        "p (r c) -> p r c", c=S3)[
                                        :, dy:dy + cnt, dx:dx + 256]
                                    nc.tensor.matmul(
                                        ps[:, :cnt * 256],
                                        w4_r[dy * 2 + dx][kt][:, mt * 128:(mt + 1) * 128],
                                        rhs, start=(i == 0), stop=(i == 15))
                                    i += 1
                        at = a4pool.tile([128, 3 * S4], f32r, tag=f"a4k{mt}")
                        a4_tiles[v].append(at)
                        av = at[:].rearrange("p (r c) -> p r c", c=S4)[:, :3, :]
                        nc.vector.tensor_copy(av[:, 0:2, 0:1], zv[:, 0:2, 0:1])
                        nc.scalar.activation(av[:, 0:cnt, 1:257],
                                             ps[:, :cnt * 256].rearrange(
                                                 "p (r c) -> p r c", c=256),
                                             PRELU, bias=b4_t[mt][:], scale=1.0,
                                             alpha=a4)
                        if v >= 1:
                            nc.vector.tensor_copy(
                                a4_tiles[v - 1][mt][:, 2 * S4:3 * S4],
                                at[:, 0:S4])

                # ---------------- L5: block t-3 -> act5 ----------------
                s = t - 3
                if 0 <= s <= 63:
                    ps = ps5pool.tile([64, 512], f32, tag="ps5")
                    i = 0
                    for dy in range(2):
                        for dx in range(2):
                            for kt in range(2):
                                rhs = a4_tiles[s][kt][:].rearrange(
                                    "p (r c) -> p r c", c=S4)[:, dy:dy + 2, dx:dx + 256]
                                nc.tensor.matmul(ps[:], w5_r[dy * 2 + dx][kt][:],
                                                 rhs, start=(i == 0), stop=(i == 7))
                                i += 1
                    a5t = a5pool.tile([64, 512], f32r, tag="a5")
                    a5_tiles[s] = a5t
                    nc.scalar.activation(a5t[:], ps[:], PRELU, bias=b5_t[:],
                                         scale=1.0, alpha=a5)

                    # ------------- L6 + mask + store: block s -------------
                    ps6 = ps6pool.tile([1, 512], f32, tag="ps6")
                    nc.tensor.matmul(ps6[:], w6_r[:], a5_tiles[s][:],
                                     start=True, stop=True)
                    mk = outpool.tile([1, 512], f32, tag="mask")
                    nc.sync.dma_start(mk[:], mask_d.ap()[:, s * 512:(s + 1) * 512])
                    ot = outpool.tile([1, 512], f32, tag="outt")
                    nc.vector.tensor_mul(ot[:], ps6[:], mk[:])
                    nc.sync.dma_start(out_d.ap()[:, s * 512:(s + 1) * 512], ot[:])

    nc.compile()
    return nc


def _host_prep(inputs):
    x = np.ascontiguousarray(np.asarray(inputs["x"], dtype=np.float32)[..., 0])
    w1 = np.asarray(inputs["w1"], np.float32)[0, 0, 0]
    w2 = np.asarray(inputs["w2"], np.float32)
    w3 = np.asarray(inputs["w3"], np.float32)
    w4 = np.asarray(inputs["w4"], np.float32)
    w5 = np.asarray(inputs["w5"], np.float32)
    w6 = np.asarray(inputs["w6"], np.float32)[0, 0, :, :]  # [64, 1]
    a1 = float(np.asarray(inputs["a1"]))
    alphas = tuple(float(np.asarray(inputs[f"a{i}"])) for i in range(2, 6))
    for bn in ("b1", "b2", "b3", "b4", "b5", "b6"):
        assert np.all(np.asarray(inputs[bn]) == 0.0), f"{bn} must be zero"

    # L1 fold: A [64, 2]; W2A [8, 256] in (dy, dx, j) partition order
    A = np.stack([np.where(w1 >= 0, w1, a1 * w1),
                  np.where(w1 >= 0, a1 * w1, w1)], axis=1).astype(np.float32)
    W2A = np.zeros((2, 2, 2, 256), np.float32)
    for dy in range(2):
        for dx in range(2):
            W2A[dy, dx] = np.einsum("co,cj->jo", w2[dy, dx], A)
    W2A = np.ascontiguousarray(W2A.reshape(8, 256))

    w3_s = np.ascontiguousarray(w3.reshape(4, 256, 512))
    w4_s = np.ascontiguousarray(w4.reshape(4, 512, 256))
    w5_s = np.ascontiguousarray(w5.reshape(4, 256, 64))
    b2 = np.asarray(inputs["b2"], np.float32).reshape(256, 1)
    b3 = np.asarray(inputs["b3"], np.float32).reshape(512, 1)
    b4 = np.asarray(inputs["b4"], np.float32).reshape(256, 1)
    b5 = np.asarray(inputs["b5"], np.float32).reshape(64, 1)

    shared = dict(w2a=W2A, w3=w3_s, w4=w4_s, w5=w5_s,
                  w6=np.ascontiguousarray(w6), b2=b2, b3=b3, b4=b4, b5=b5)

    in_maps = []
    for b in range(B):
        xb = x[b]
        V = np.stack([np.maximum(xb, 0), np.minimum(xb, 0)])   # [2, 256, 256]
        V_ext = np.zeros((2, 264, 257), np.float32)
        V_ext[:, 4:260, :256] = V
        for h in range(2):
            S = h * HALF + 2
            V_win = V_ext[:, S:S + XROWS, :]
            Vst = np.zeros((2, 2, 2, VROWS, 256), np.float32)
            for dy in range(2):
                for dx in range(2):
                    for j in range(2):
                        Vst[dy, dx, j] = V_win[j, dy:dy + VROWS, dx:dx + 256]
            Vst = Vst.reshape(8, VROWS * 256)
            rows_g = np.arange(h * HALF, h * HALF + HALF)[:, None]
            cols_g = np.arange(256)[None, :]
            mask = (rows_g >= cols_g).astype(np.float32).reshape(1, HALF * 256)
            in_maps.append({"vstack": np.ascontiguousarray(Vst), "mask": mask,
                            **shared})
    return in_maps, alphas


def kernel(**inputs):
    global LAST_RESULTS
    in_maps, alphas = _host_prep(inputs)
    key = alphas
    if key not in _CACHE:
        _CACHE[key] = _build_program(alphas)
    nc = _CACHE[key]
    res = run_bass_kernel_spmd(nc, in_maps, core_ids=list(range(8)))
    LAST_RESULTS = res
    out = np.zeros((B, N, N, 1), np.float32)
    ci = 0
    for b in range(B):
        for h in range(2):
            out[b, h * HALF:(h + 1) * HALF, :, 0] = (
                res.results[ci]["out"].reshape(HALF, 256))
            ci += 1
    return out


# revision 15
# speedup vs baseline: 46.5743x; 46.5743x over previous
"""Trainium2 Bass kernel for nn_PreconditionerNet (5-layer conv stack + PReLU + tril mask).

Strategy
--------
8 cores = 4 images x 2 row-halves (halo recompute, no collectives). Each core
runs an identical program over a 132-row input window and produces 128 rows of
the final 256x256 output for its (image, half).

Inside a core the whole conv stack is fused into one pass over 2-row "ticks":
activations live only in SBUF rolling windows (3-row tiles), so DRAM traffic is
just the tiny input window + output. Convs are computed as fp32r matmuls
(channels on partitions, 2x2 kernels = 4 shifted accumulating matmuls); PReLU
epilogues run on the scalar engine as single Prelu ops (PSUM -> SBUF).

conv1 (1x1, 1->64) + PReLU is folded into conv2 on the host: with b1 == 0,
PReLU(w1_c * x) = A[c,0]*relu(x) + A[c,1]*min(x,0), so conv2 over the 64
PReLU'd channels becomes a K=8 matmul over a host-built "Vstack" tensor
(4 spatial shifts x 2 half-rectified channels) with folded weights W2A.
"""

import os
import sys

sys.path.insert(0, "/opt/trn_rl_repo")

# NTFF tracing hooks are unavailable in this container; make sure
# run_bass_kernel_spmd never tries to trace even if BASS_TRACE is set.
os.environ["BASS_NEVER_TRACE"] = "1"

import numpy as np

import concourse.bacc as bacc
import concourse.mybir as mybir
import concourse.tile as tile
import concourse.bass_utils as _bass_utils
from concourse.bass_utils import run_bass_kernel_spmd

# Enable walrus LDWEIGHTS optimization: the default --enable-ldw-opt=false
# makes every fp32r matmul pay a serialized weight reload (~70-95 ns/mm,
# ~20% of this kernel's runtime). Correctness is validated against the
# reference with the flag on.
if not getattr(_bass_utils, "_ldw_opt_patched", False):
    _orig_run_command = _bass_utils.run_command

    def _run_command_ldw_opt(cmd, **kw):
        if isinstance(cmd, list):
            cmd = [c.replace("--enable-ldw-opt=false", "--enable-ldw-opt=true")
                   if isinstance(c, str) else c for c in cmd]
        return _orig_run_command(cmd, **kw)

    _bass_utils.run_command = _run_command_ldw_opt
    _bass_utils._ldw_opt_patched = True

f32 = mybir.dt.float32
f32r = mybir.dt.float32r
PRELU = mybir.ActivationFunctionType.Prelu

# Geometry (hardcoded for x: [4, 256, 256, 1])
B, N = 4, 256
HALF = 128
XROWS = 132          # input window rows per core
VROWS = 131          # Vstack rows (= L2 blocks * 2 - 1)
R2, R3, R4, R5 = 131, 130, 129, 128   # act rows per layer window
T2 = 66              # L2 2-row blocks (last is single-row)
S2, S3, S4 = 256, 258, 258            # stored column strides (act2/act3/act4)
NTICKS = 67

_CACHE = {}
LAST_RESULTS = None


def _build_program(alphas, repeat=1):
    a2, a3, a4, a5 = alphas
    nc = bacc.Bacc("TRN2", target_bir_lowering=False, debug=False)

    # --- DRAM tensors ---
    vst_d = nc.dram_tensor("vstack", [8, VROWS * 256], f32, kind="ExternalInput")
    mask_d = nc.dram_tensor("mask", [1, HALF * 256], f32, kind="ExternalInput")
    w2a_d = nc.dram_tensor("w2a", [8, 256], f32, kind="ExternalInput")
    w3_d = nc.dram_tensor("w3", [4, 256, 512], f32, kind="ExternalInput")
    w4_d = nc.dram_tensor("w4", [4, 512, 256], f32, kind="ExternalInput")
    w5_d = nc.dram_tensor("w5", [4, 256, 64], f32, kind="ExternalInput")
    w6_d = nc.dram_tensor("w6", [64, 1], f32, kind="ExternalInput")
    b2_d = nc.dram_tensor("b2", [256, 1], f32, kind="ExternalInput")
    b3_d = nc.dram_tensor("b3", [512, 1], f32, kind="ExternalInput")
    b4_d = nc.dram_tensor("b4", [256, 1], f32, kind="ExternalInput")
    b5_d = nc.dram_tensor("b5", [64, 1], f32, kind="ExternalInput")
    out_d = nc.dram_tensor("out", [1, HALF * 256], f32, kind="ExternalOutput")

    with tile.TileContext(nc) as tc:
        with (
            tc.tile_pool(name="wstage", bufs=2) as wstage,
            tc.tile_pool(name="weights", bufs=1) as wpool,
            tc.tile_pool(name="vs", bufs=3) as vspool,
            tc.tile_pool(name="a2", bufs=3) as a2pool,
            tc.tile_pool(name="a3", bufs=3) as a3pool,
            tc.tile_pool(name="a4", bufs=3) as a4pool,
            tc.tile_pool(name="a5", bufs=3) as a5pool,
            tc.tile_pool(name="outst", bufs=3) as outpool,
            tc.tile_pool(name="ps2", bufs=2, space="PSUM") as ps2pool,
            tc.tile_pool(name="ps3", bufs=2, space="PSUM") as ps3pool,
            tc.tile_pool(name="ps4", bufs=2, space="PSUM") as ps4pool,
            tc.tile_pool(name="ps5", bufs=1, space="PSUM") as ps5pool,
            tc.tile_pool(name="ps6", bufs=1, space="PSUM") as ps6pool,
        ):
            # ---- load + convert weights to f32r ----
            def load_wr(src_ap, p, cols, tag):
                st = wstage.tile([128, 512], f32, tag="wstage")
                nc.sync.dma_start(st[:p, :cols], src_ap)
                wr = wpool.tile([p, cols], f32r, tag=tag)
                nc.vector.tensor_copy(wr[:], st[:p, :cols])
                return wr

            w2a_r = load_wr(w2a_d.ap(), 8, 256, "w2a")
            w3_r = [[load_wr(w3_d.ap()[s, kt * 128:(kt + 1) * 128, :], 128, 512,
                             f"w3_{s}_{kt}") for kt in range(2)] for s in range(4)]
            w4_r = [[load_wr(w4_d.ap()[s, kt * 128:(kt + 1) * 128, :], 128, 256,
                             f"w4_{s}_{kt}") for kt in range(4)] for s in range(4)]
            w5_r = [[load_wr(w5_d.ap()[s, kt * 128:(kt + 1) * 128, :], 128, 64,
                             f"w5_{s}_{kt}") for kt in range(2)] for s in range(4)]
            w6_r = load_wr(w6_d.ap(), 64, 1, "w6")

            def load_bias(src_ap, p, tag):
                bt = wpool.tile([p, 1], f32, tag=tag)
                nc.sync.dma_start(bt[:], src_ap)
                return bt

            b2_t = [load_bias(b2_d.ap()[mt * 128:(mt + 1) * 128, :], 128, f"b2_{mt}")
                    for mt in range(2)]
            b3_t = [load_bias(b3_d.ap()[mt * 128:(mt + 1) * 128, :], 128, f"b3_{mt}")
                    for mt in range(4)]
            b4_t = [load_bias(b4_d.ap()[mt * 128:(mt + 1) * 128, :], 128, f"b4_{mt}")
                    for mt in range(2)]
            b5_t = load_bias(b5_d.ap(), 64, "b5")

            # persistent zeros tile (f32r) for margin zeroing
            zf = wpool.tile([128, 8], f32, tag="zf")
            nc.vector.memset(zf[:], 0.0)
            zr = wpool.tile([128, 8], f32r, tag="zr")
            nc.vector.tensor_copy(zr[:], zf[:])
            zv = zr[:].rearrange("p (r c) -> p r c", c=4)  # [128, 2, 4]

            def rows(view):
                return view.rearrange("p (r c) -> p r c", r=3)

            for _rep in range(repeat):
              # rolling-window tiles, keyed by block index
              a2_tiles, a3_tiles, a4_tiles, a5_tiles = {}, {}, {}, {}
              for t in range(NTICKS):
                # ---------------- L2: block t -> act2 ----------------
                if t < T2:
                    cnt = 2 if t < T2 - 1 else 1
                    vs_f = vspool.tile([8, 512], f32, tag="vsf")
                    nc.sync.dma_start(
                        vs_f[:, :cnt * 256],
                        vst_d.ap()[:, 2 * t * 256:(2 * t + cnt) * 256])
                    vs_r = vspool.tile([8, 512], f32r, tag="vsr")
                    nc.vector.tensor_copy(vs_r[:, :cnt * 256], vs_f[:, :cnt * 256])
                    a2_tiles[t] = []
                    for mt in range(2):
                        ps = ps2pool.tile([128, 512], f32, tag="ps2")
                        nc.tensor.matmul(ps[:, :cnt * 256],
                                         w2a_r[:, mt * 128:(mt + 1) * 128],
                                         vs_r[:, :cnt * 256],
                                         start=True, stop=True)
                        at = a2pool.tile([128, 3 * S2], f32r, tag=f"a2k{mt}")
                        a2_tiles[t].append(at)
                        nc.scalar.activation(at[:, :cnt * 256], ps[:, :cnt * 256],
                                             PRELU, bias=b2_t[mt][:], scale=1.0,
                                             alpha=a2)
                        if t >= 1:
                            # duplicate first row into previous tile's row 2
                            nc.vector.tensor_copy(
                                a2_tiles[t - 1][mt][:, 2 * S2:2 * S2 + 256],
                                at[:, 0:256])

                # ---------------- L3: block t-1 -> act3 ----------------
                u = t - 1
                if 0 <= u <= 64:
                    a3_tiles[u] = []
                    for mt in range(4):
                        ps = ps3pool.tile([128, 508], f32, tag="ps3")
                        i = 0
                        for dy in range(2):
                            for dx in range(2):
                                for kt in range(2):
                                    rhs = rows(a2_tiles[u][kt][:])[
                                        :, dy:dy + 2, dx:dx + 254]
                                    nc.tensor.matmul(
                                        ps[:],
                                        w3_r[dy * 2 + dx][kt][:, mt * 128:(mt + 1) * 128],
                                        rhs, start=(i == 0), stop=(i == 7))
                                    i += 1
                        at = a3pool.tile([128, 3 * S3], f32r, tag=f"a3k{mt}")
                        a3_tiles[u].append(at)
                        av = at[:].rearrange("p (r c) -> p r c", c=S3)
                        # zero margins (cols 0, 255, 256) for rows 0..1
                        nc.vector.tensor_copy(av[:, 0:2, 0:1], zv[:, 0:2, 0:1])
                        nc.vector.tensor_copy(av[:, 0:2, 255:257], zv[:, 0:2, 0:2])
                        nc.scalar.activation(av[:, 0:2, 1:255],
                                             ps[:].rearrange("p (r c) -> p r c", c=254),
                                             PRELU, bias=b3_t[mt][:], scale=1.0,
                                             alpha=a3)
                        if u >= 1:
                            nc.vector.tensor_copy(
                                a3_tiles[u - 1][mt][:, 2 * S3:3 * S3],
                                at[:, 0:S3])

                # ---------------- L4: block t-2 -> act4 ----------------
                v = t - 2
                if 0 <= v <= 64:
                    cnt = 2 if v < 64 else 1
                    a4_tiles[v] = []
                    for mt in range(2):
                        ps = ps4pool.tile([128, 512], f32, tag="ps4")
                        i = 0
                        for dy in range(2):
                            for dx in range(2):
                                for kt in range(4):
                                    rhs = a3_tiles[v][kt][:].rearrange(
                                        "p (r c) -> p r c", c=S3)[
                                        :, dy:dy + cnt, dx:dx + 256]
                                    nc.tensor.matmul(
                                        ps[:, :cnt * 256],
                                        w4_r[dy * 2 + dx][kt][:, mt * 128:(mt + 1) * 128],
                                        rhs, start=(i == 0), stop=(i == 15))
                                    i += 1
                        at = a4pool.tile([128, 3 * S4], f32r, tag=f"a4k{mt}")
                        a4_tiles[v].append(at)
                        av = at[:].rearrange("p (r c) -> p r c", c=S4)[:, :3, :]
                        nc.vector.tensor_copy(av[:, 0:2, 0:1], zv[:, 0:2, 0:1])
                        nc.scalar.activation(av[:, 0:cnt, 1:257],
                                             ps[:, :cnt * 256].rearrange(
                                                 "p (r c) -> p r c", c=256),
                                             PRELU, bias=b4_t[mt][:], scale=1.0,
                                             alpha=a4)
                        if v >= 1:
                            nc.vector.tensor_copy(
                                a4_tiles[v - 1][mt][:, 2 * S4:3 * S4],
                                at[:, 0:S4])

                # ---------------- L5: block t-3 -> act5 ----------------
                s = t - 3
                if 0 <= s <= 63:
                    ps = ps5pool.tile([64, 512], f32, tag="ps5")
                    i = 0
                    for dy in range(2):
                        for dx in range(2):
                            for kt in range(2):
                                rhs = a4_tiles[s][kt][:].rearrange(
                                    "p (r c) -> p r c", c=S4)[:, dy:dy + 2, dx:dx + 256]
                                nc.tensor.matmul(ps[:], w5_r[dy * 2 + dx][kt][:],
                                                 rhs, start=(i == 0), stop=(i == 7))
                                i += 1
                    a5t = a5pool.tile([64, 512], f32r, tag="a5")
                    a5_tiles[s] = a5t
                    nc.scalar.activation(a5t[:], ps[:], PRELU, bias=b5_t[:],
                                         scale=1.0, alpha=a5)

                    # ------------- L6 + mask + store: block s -------------
                    ps6 = ps6pool.tile([1, 512], f32, tag="ps6")
                    nc.tensor.matmul(ps6[:], w6_r[:], a5_tiles[s][:],
                                     start=True, stop=True)
                    mk = outpool.tile([1, 512], f32, tag="mask")
                    nc.sync.dma_start(mk[:], mask_d.ap()[:, s * 512:(s + 1) * 512])
                    ot = outpool.tile([1, 512], f32, tag="outt")
                    nc.vector.tensor_mul(ot[:], ps6[:], mk[:])
                    nc.sync.dma_start(out_d.ap()[:, s * 512:(s + 1) * 512], ot[:])

    nc.compile()
    return nc


def _host_prep(inputs):
    x = np.ascontiguousarray(np.asarray(inputs["x"], dtype=np.float32)[..., 0])
    w1 = np.asarray(inputs["w1"], np.float32)[0, 0, 0]
    w2 = np.asarray(inputs["w2"], np.float32)
    w3 = np.asarray(inputs["w3"], np.float32)
    w4 = np.asarray(inputs["w4"], np.float32)
    w5 = np.asarray(inputs["w5"], np.float32)
    w6 = np.asarray(inputs["w6"], np.float32)[0, 0, :, :]  # [64, 1]
    a1 = float(np.asarray(inputs["a1"]))
    alphas = tuple(float(np.asarray(inputs[f"a{i}"])) for i in range(2, 6))
    for bn in ("b1", "b2", "b3", "b4", "b5", "b6"):
        assert np.all(np.asarray(inputs[bn]) == 0.0), f"{bn} must be zero"

    # L1 fold: A [64, 2]; W2A [8, 256] in (dy, dx, j) partition order
    A = np.stack([np.where(w1 >= 0, w1, a1 * w1),
                  np.where(w1 >= 0, a1 * w1, w1)], axis=1).astype(np.float32)
    W2A = np.zeros((2, 2, 2, 256), np.float32)
    for dy in range(2):
        for dx in range(2):
            W2A[dy, dx] = np.einsum("co,cj->jo", w2[dy, dx], A)
    W2A = np.ascontiguousarray(W2A.reshape(8, 256))

    w3_s = np.ascontiguousarray(w3.reshape(4, 256, 512))
    w4_s = np.ascontiguousarray(w4.reshape(4, 512, 256))
    w5_s = np.ascontiguousarray(w5.reshape(4, 256, 64))
    b2 = np.asarray(inputs["b2"], np.float32).reshape(256, 1)
    b3 = np.asarray(inputs["b3"], np.float32).reshape(512, 1)
    b4 = np.asarray(inputs["b4"], np.float32).reshape(256, 1)
    b5 = np.asarray(inputs["b5"], np.float32).reshape(64, 1)

    shared = dict(w2a=W2A, w3=w3_s, w4=w4_s, w5=w5_s,
                  w6=np.ascontiguousarray(w6), b2=b2, b3=b3, b4=b4, b5=b5)

    in_maps = []
    for b in range(B):
        xb = x[b]
        V = np.stack([np.maximum(xb, 0), np.minimum(xb, 0)])   # [2, 256, 256]
        V_ext = np.zeros((2, 264, 257), np.float32)
        V_ext[:, 4:260, :256] = V
        for h in range(2):
            S = h * HALF + 2
            V_win = V_ext[:, S:S + XROWS, :]
            Vst = np.zeros((2, 2, 2, VROWS, 256), np.float32)
            for dy in range(2):
                for dx in range(2):
                    for j in range(2):
                        Vst[dy, dx, j] = V_win[j, dy:dy + VROWS, dx:dx + 256]
            Vst = Vst.reshape(8, VROWS * 256)
            rows_g = np.arange(h * HALF, h * HALF + HALF)[:, None]
            cols_g = np.arange(256)[None, :]
            mask = (rows_g >= cols_g).astype(np.float32).reshape(1, HALF * 256)
            in_maps.append({"vstack": np.ascontiguousarray(Vst), "mask": mask,
                            **shared})
    return in_maps, alphas


def kernel(**inputs):
    global LAST_RESULTS
    in_maps, alphas = _host_prep(inputs)
    key = alphas
    if key not in _CACHE:
        _CACHE[key] = _build_program(alphas)
    nc = _CACHE[key]
    res = run_bass_kernel_spmd(nc, in_maps, core_ids=list(range(8)))
    LAST_RESULTS = res
    out = np.zeros((B, N, N, 1), np.float32)
    ci = 0
    for b in range(B):
        for h in range(2):
            out[b, h * HALF:(h + 1) * HALF, :, 0] = (
                res.results[ci]["out"].reshape(HALF, 256))
            ci += 1
    return out
